# revision 24
# baseline (speedup 1.0000x reference)
"""AgentCollisionLoss Trainium2 kernel.

Full inputs -> full output. Shards the N (sample) dim across 8 NeuronCores
(2 samples per core), computes the pairwise agent-collision loss on device,
and gathers the per-core (NL, B) losses into the full (B, N) output.

Device layout (per core):
  partition p = n_local*T + t            (104 rows)
  Stage A: world-frame disk centroids CXY [P, 2*B*D] from x + per-agent consts
  Stage B: per scene block, outer-difference over the packed disk-point list,
           squares on ACT, add + two-stage min-reduce over (dj, di) on DVE
  Stage C: sqrt, penalty = relu(1 - dist/pd) on the packed pair list
  Stage D: time-decay-weighted sum over t via a [P,2]^T @ [P,32] matmul,
           moving-mask, DMA out [2, 32]

All broadcast constants ride in the xin tensor (replicated per partition on
the host) so the kernel issues a single big input DMA.
"""

import os
import sys

import numpy as np

for _p in ("/opt/trn_rl_repo", "/root/.axon_site/_ro/trn_rl_repo"):
    if os.path.isdir(_p) and _p not in sys.path:
        sys.path.insert(0, _p)

import bass_rust
import concourse.bass as bass
import concourse.mybir as mybir
import concourse.tile as tile
from concourse.bass_utils import run_bass_kernel_spmd
from concourse.vector_clock import ScopedClock


def _split_drain_and_barrier(self, tick_clock, wait_clock):
    """Kernel-tail drain, one semaphore per drain instruction.

    The walrus build in this container rejects instructions carrying more
    than one embedded sync wait ("Too many sync wait commands"). Tile's
    stock tail emits a single drain waiting on the full global clock, so
    split it: one drain per nonzero proc tick. add_sem_waits elides waits
    the engine has already observed, so each drain carries exactly one.
    """
    gc = list(tick_clock.global_clock)
    for idx, tick in enumerate(gc):
        if tick <= 0:
            continue
        v = [0] * len(gc)
        v[idx] = tick
        d = self.nc.sync.drain()
        wait_clock.add_sem_waits(
            d.ins, ScopedClock({None: bass_rust.VectorClock(v)})
        )
    self.nc.all_engine_barrier()
    assert self.sems is not None
    popped = self.nc._tile_sem_poison_stack.pop()
    assert popped is self._sem_poison
    self.nc.clear_and_free_semaphores(list(self.sems.allocated().values()))
    self.nc.all_engine_barrier()


tile.TileContext._drain_and_barrier = _split_drain_and_barrier

B, N, T, D = 32, 16, 52, 5
NCORES = 8
NL = N // NCORES          # samples per core
P = NL * T                # partition rows per core
BUFFER_DIST = 0.2
DECAY_RATE = 0.9
SPEED_TH = 0.5
FMAX = 4000               # max free elems per big-stage chunk

F32 = mybir.dt.float32
F16 = mybir.dt.float16
PI = float(np.pi)

# bulk dtype for squared distances (precision analysed: d2 < 43000 < f16 max,
# and only d2 <= pd^2 ~ 16 matters, where f16 ulp ~ 0.008-0.016)
DT_BULK = F16
# chunk indices (descending-size order) whose outer-difference runs on
# gpsimd instead of DVE (gpsimd is ~2.3x slower per element but otherwise
# idle; offloading mid-size chunks rebalances the engines)
SUB_ON_GPSIMD = (1, 2)


def _chunks(scenes):
    """[(scene_off, ci, sc, s), ...] i-chunks, largest first."""
    out = []
    for (o, s) in scenes:
        sc_max = max(1, FMAX // (s * D * D))
        ci = 0
        while ci < s:
            sc = min(sc_max, s - ci)
            out.append((o, ci, sc, s))
            ci += sc
    out.sort(key=lambda c: -(c[2] * c[3]))
    return out


# xin column layout (offsets in f32 elements)
XO_X = 0                  # 3*B per-partition x data (x0 | x1 | yaw)
XO_GEO = 3 * B            # gA(2B) gB(2B) gT(2B) shifts2(2B) = 8B
XO_CENT = XO_GEO + 8 * B  # B*D
XO_MVR = XO_CENT + B * D  # 2B (moving mask, NL copies)
XO_PRC = XO_MVR + 2 * B   # PP
# then wmt (NL cols)


def _xin_width(PP):
    return XO_PRC + PP + NL


def _build_nc(scenes, PP):
    """Build the SPMD Bass program. `scenes` = [(offset, size)], PP = sum s^2."""
    nc = bass.Bass()

    XW = _xin_width(PP)
    xin = nc.dram_tensor("xin", [P, XW], F32, kind="ExternalInput")
    out = nc.dram_tensor("loss", [NL, B], F32, kind="ExternalOutput")

    chunks = _chunks(scenes)

    with tile.TileContext(nc) as tc:
        with (
            tc.tile_pool(name="singles", bufs=1) as singles,
            tc.tile_pool(name="small", bufs=1) as small,
            tc.tile_pool(name="big", bufs=1) as big,
            tc.tile_pool(name="psum", bufs=1, space="PSUM") as psum,
        ):
            # ---- loads ----
            xt = singles.tile([P, XW], F32)
            nc.sync.dma_start(out=xt[:], in_=xin[:])
            ones = singles.tile([P, 1], F32)
            nc.vector.memset(ones[:], 1.0)

            # Pre-touch the DMA'd tile on DVE: the copy carries the one
            # DMA-queue sem wait, so later compute ops joining DMA data with
            # engine-produced data need at most one new wait (this walrus
            # rejects instructions with more than one embedded sync wait).
            tch = singles.tile([P, 1], F32, tag="tch0")
            nc.vector.tensor_copy(out=tch[:], in_=xt[:, 0:1])
            # matmul weights via DVE so the PE matmul's deps are DVE-only
            wt2 = singles.tile([P, NL], F32)
            nc.vector.tensor_copy(out=wt2[:], in_=xt[:, XO_PRC + PP : XO_PRC + PP + NL])

            gA = xt[:, XO_GEO + 0 * B : XO_GEO + 2 * B]
            gB = xt[:, XO_GEO + 2 * B : XO_GEO + 4 * B]
            gT = xt[:, XO_GEO + 4 * B : XO_GEO + 6 * B]
            shifts2 = xt[:, XO_GEO + 6 * B : XO_GEO + 8 * B]
            x0 = xt[:, 0:B]
            x1 = xt[:, B : 2 * B]
            yw = xt[:, 2 * B : 3 * B]
            cxc = xt[:, XO_CENT : XO_CENT + B * D]
            movt = xt[0:NL, XO_MVR : XO_MVR + B]   # replicated const rows
            prc = xt[:, XO_PRC : XO_PRC + PP]

            def rep2(apx, w):
                """view [P, 2, w] reading apx's first w elems twice"""
                return bass.AP(tensor=apx.tensor, offset=apx.offset,
                               ap=[apx.ap[0], [0, 2], [1, w]])

            # ---- stage A ----
            # u = yaw/2pi + (shift + yoff/2pi)   (shift 2.0 -> sin, 2.25 -> cos)
            u2 = small.tile([P, 2, B], F32)
            nc.vector.scalar_tensor_tensor(
                out=u2[:], in0=rep2(yw, B), scalar=1.0 / (2.0 * PI),
                in1=shifts2.rearrange("p (c i) -> p c i", c=2),
                op0=mybir.AluOpType.mult, op1=mybir.AluOpType.add)
            # round-to-nearest-even via the 1.5*2^23 magic constant
            MAGIC = 12582912.0
            kf = small.tile([P, 2, B], F32)
            nc.vector.tensor_scalar(
                out=kf[:], in0=u2[:], scalar1=MAGIC, scalar2=MAGIC,
                op0=mybir.AluOpType.add, op1=mybir.AluOpType.subtract)
            fr = small.tile([P, 2, B], F32)
            nc.vector.tensor_sub(fr[:], u2[:], kf[:])
            # sincos[:, 0:32] = sin(yawg), [:, 32:64] = cos(yawg)
            sincos = small.tile([P, 2 * B], F32)
            nc.scalar.activation(out=sincos[:].rearrange("p (c i) -> p c i", c=2),
                                 in_=fr[:],
                                 func=mybir.ActivationFunctionType.Sin,
                                 bias=0.0, scale=2.0 * PI)

            # pos_g for both coords: pg[p, c, i], c=0 -> x, 1 -> y
            m1 = small.tile([P, 2, B], F32)
            m2 = small.tile([P, 2, B], F32)
            pg = small.tile([P, 2, B], F32)
            nc.vector.tensor_mul(m1[:], rep2(x0, B),
                                 gA.rearrange("p (c i) -> p c i", c=2))
            nc.vector.tensor_mul(m2[:], rep2(x1, B),
                                 gB.rearrange("p (c i) -> p c i", c=2))
            nc.vector.tensor_add(pg[:], m1[:], m2[:])
            nc.vector.tensor_add(pg[:], pg[:],
                                 gT.rearrange("p (c i) -> p c i", c=2))

            # CXY[p, c, i, di] = cent_x(i,di) * cs(c,i) + pg(c,i)
            # c=0 uses cos, c=1 uses sin (x = cx*cos + pgx, y = cx*sin + pgy)
            cxy = singles.tile([P, 2, B, D], F32)
            cs_sel = bass.AP(tensor=sincos.tensor, offset=sincos[:].offset + B,
                             ap=[sincos[:].ap[0], [-B, 2], [1, B], [0, D]])
            cx_rep = bass.AP(tensor=xt.tensor, offset=cxc.offset,
                             ap=[cxc.ap[0], [0, 2], [D, B], [1, D]])
            pg_bc = bass.AP(tensor=pg.tensor, offset=pg[:].offset,
                            ap=[pg[:].ap[0], [B, 2], [1, B], [0, D]])
            nc.vector.tensor_mul(cxy[:], cx_rep, cs_sel)
            nc.vector.tensor_add(cxy[:], cxy[:], pg_bc)

            cxyf = cxy[:].rearrange("p c i d -> p (c i d)")
            pap = cxyf.ap[0]
            e = cxyf.ap[-1][0]

            # ---- stage B ----
            pdist = singles.tile([P, PP], F32)
            NPTS = B * D

            def emit_subs(idx):
                (o, ci, sc, s) = chunks[idx]
                m, q = D * sc, D * s
                # one sub for both coords: (c, m, q) with A bcast over q,
                # B bcast over m
                a_ap = bass.AP(tensor=cxyf.tensor,
                               offset=cxyf.offset + (o + ci) * D * e,
                               ap=[pap, [NPTS * e, 2], [e, m], [0, q]])
                b_ap = bass.AP(tensor=cxyf.tensor,
                               offset=cxyf.offset + o * D * e,
                               ap=[pap, [NPTS * e, 2], [0, m], [e, q]])
                sub = big.tile([P, 2, m, q], DT_BULK, tag=f"sub{idx}")
                sub_eng = nc.gpsimd if idx in SUB_ON_GPSIMD else nc.vector
                sub_eng.tensor_tensor(out=sub[:], in0=a_ap, in1=b_ap,
                                      op=mybir.AluOpType.subtract)
                return sub

            def emit_squares(idx, sub):
                (o, ci, sc, s) = chunks[idx]
                m, q = D * sc, D * s
                # two squares (separate tiles keep the d2-add operands
                # tile-aligned for the fp16 2x mode)
                sq = {}
                for c, nm in ((0, "x"), (1, "y")):
                    tsq = big.tile([P, m, q], DT_BULK, tag=f"sq{nm}{idx}")
                    nc.scalar.activation(
                        out=tsq[:].rearrange("p a b -> p (a b)"),
                        in_=sub[:, c, :, :].rearrange("p a b -> p (a b)"),
                        func=mybir.ActivationFunctionType.Square)
                    sq[nm] = tsq
                return sq

            def emit_tail(idx, sq, poff):
                (o, ci, sc, s) = chunks[idx]
                m, q = D * sc, D * s
                d2 = big.tile([P, m, q], DT_BULK, tag=f"d2{idx}")
                add_eng = nc.vector
                add_eng.tensor_tensor(
                    out=d2[:].rearrange("p a b -> p (a b)"),
                    in0=sq["x"][:].rearrange("p a b -> p (a b)"),
                    in1=sq["y"][:].rearrange("p a b -> p (a b)"),
                    op=mybir.AluOpType.add)
                # min over dj: view (m, j, dj), innermost dj; scatter-write
                # r1 in (i, j, di) order so the second reduce reads unit-stride
                r1 = big.tile([P, sc, s, D], DT_BULK, tag=f"r1{idx}")
                r1f = r1[:].rearrange("p a b c -> p (a b c)")
                e1 = r1f.ap[-1][0]
                r1scat = bass.AP(tensor=r1f.tensor, offset=r1f.offset,
                                 ap=[r1f.ap[0], [s * D * e1, sc], [e1, D],
                                     [D * e1, s]])
                nc.vector.tensor_reduce(
                    out=r1scat,
                    in_=d2[:].rearrange("p a (j dj) -> p a j dj", dj=D),
                    axis=mybir.AxisListType.X, op=mybir.AluOpType.min)
                pmin = pdist[:, poff : poff + sc * s].rearrange(
                    "p (a b) -> p a b", b=s)
                nc.vector.tensor_reduce(out=pmin, in_=r1[:],
                                        axis=mybir.AxisListType.X,
                                        op=mybir.AluOpType.min)

            # emission: all subs first (DVE and gpsimd queues fill in
            # parallel), then squares and tails in data-availability order
            # (DVE-sub chunks first, gpsimd-sub chunks after)
            poffs = []
            po = 0
            for (o, ci, sc, s) in chunks:
                poffs.append(po)
                po += sc * s
            assert po == PP
            order = [i for i in range(len(chunks)) if i not in SUB_ON_GPSIMD] \
                + [i for i in range(len(chunks)) if i in SUB_ON_GPSIMD]
            subs = {}
            for idx in range(len(chunks)):
                subs[idx] = emit_subs(idx)
            sqs = {}
            for idx in order:
                sqs[idx] = emit_squares(idx, subs[idx])
            for idx in order:
                emit_tail(idx, sqs[idx], poffs[idx])

            # ---- stage C ----
            dist = small.tile([P, PP], F32, tag="dist")
            nc.scalar.activation(out=dist[:], in_=pdist[:],
                                 func=mybir.ActivationFunctionType.Sqrt)
            rr = small.tile([P, PP], F32, tag="rr")
            nc.vector.tensor_mul(rr[:], dist[:], prc)
            # pen = relu(1 - r)
            pen = small.tile([P, PP], F32, tag="pen")
            nc.scalar.activation(out=pen[:], in_=rr[:],
                                 func=mybir.ActivationFunctionType.Relu,
                                 bias=ones[:], scale=-1.0)

            # ---- j-sums per chunk row-block -> loss32 [P, B] ----
            loss32 = singles.tile([P, B], F32)
            for idx, (o, ci, sc, s) in enumerate(chunks):
                pv = pen[:, poffs[idx] : poffs[idx] + sc * s].rearrange(
                    "p (a b) -> p a b", b=s)
                nc.vector.tensor_reduce(out=loss32[:, o + ci : o + ci + sc],
                                        in_=pv,
                                        axis=mybir.AxisListType.X,
                                        op=mybir.AluOpType.add)

            # ---- stage D ----
            ps = psum.tile([NL, B], F32)
            nc.tensor.matmul(ps[:], wt2[:], loss32[:], start=True, stop=True)
            lout = small.tile([NL, B], F32, tag="lout")
            nc.vector.tensor_mul(lout[:], ps[:], movt[:])
            nc.sync.dma_start(out=out[:], in_=lout[:])

    return nc


def _prepare(inputs):
    x = np.ascontiguousarray(inputs["x"], dtype=np.float32)
    extent = np.asarray(inputs["extent"], dtype=np.float32)
    wfa = np.asarray(inputs["world_from_agent"], dtype=np.float32)
    speed = np.asarray(inputs["curr_speed"], dtype=np.float32)
    scene = np.asarray(inputs["scene_index"])

    R = wfa[:, :2, :2]
    tr = wfa[:, :2, 2]
    yaw_off = np.arctan2(R[:, 1, 0], R[:, 0, 0]).astype(np.float32)
    agt_rad = extent[:, 1] / 2.0
    cent_min = -(extent[:, 0] / 2.0) + agt_rad
    cent_max = (extent[:, 0] / 2.0) - agt_rad
    lin = np.linspace(0.0, 1.0, D, dtype=np.float32)
    cent_x = (cent_min[:, None] + (cent_max - cent_min)[:, None] * lin).astype(
        np.float32)
    pd = (agt_rad[:, None] + agt_rad[None, :] + BUFFER_DIST).astype(np.float32)
    moving = (np.abs(speed) > SPEED_TH)

    # contiguous scene blocks (scene_index is sorted)
    _, starts, counts = np.unique(scene, return_index=True, return_counts=True)
    scenes = [(int(o), int(s)) for o, s in zip(starts, counts)]
    assert sum(s for _, s in scenes) == B
    for o, s in scenes:
        assert (scene[o : o + s] == scene[o]).all()

    chunks = _chunks(scenes)
    pairs_i = []
    pairs_j = []
    for (o, ci, sc, s) in chunks:
        for ii in range(o + ci, o + ci + sc):
            for jj in range(o, o + s):
                pairs_i.append(ii)
                pairs_j.append(jj)
    pairs_i = np.array(pairs_i)
    pairs_j = np.array(pairs_j)
    PP = len(pairs_i)
    inv_pd = (1.0 / pd[pairs_i, pairs_j]).astype(np.float32)

    twopi = 2.0 * np.pi
    geo = np.concatenate([
        R[:, 0, 0], R[:, 1, 0],          # gA
        R[:, 0, 1], R[:, 1, 1],          # gB
        tr[:, 0], tr[:, 1],              # gT
        2.0 + yaw_off / twopi, 2.25 + yaw_off / twopi,  # shifts2
    ]).astype(np.float32)

    w = DECAY_RATE ** np.arange(T, dtype=np.float32)
    w = w / w.sum()
    wmt = np.zeros((P, NL), dtype=np.float32)
    for nl in range(NL):
        wmt[nl * T : (nl + 1) * T, nl] = w / B

    # packed xin: per-partition x data + replicated consts + wmt
    XW = _xin_width(PP)
    mvr2 = np.tile(moving.astype(np.float32), NL)
    const_row = np.concatenate([geo, cent_x.reshape(-1), mvr2, inv_pd])
    in_maps = []
    for c in range(NCORES):
        xs = x[:, c * NL : (c + 1) * NL, :, :]          # (B, NL, T, 6)
        xs = xs[..., [0, 1, 3]]                          # (B, NL, T, 3)
        xdat = xs.transpose(1, 2, 3, 0).reshape(P, 3 * B)
        xin = np.empty((P, XW), dtype=np.float32)
        xin[:, 0 : 3 * B] = xdat
        xin[:, XO_GEO : XO_PRC + PP] = const_row[None, :]
        xin[:, XO_PRC + PP :] = wmt
        in_maps.append({"xin": xin})

    return scenes, PP, in_maps, moving


_CACHE = {}


def _get_nc(scenes, PP):
    key = (tuple(scenes), PP)
    if key not in _CACHE:
        _CACHE[key] = _build_nc(scenes, PP)
    return _CACHE[key]


def _run(inputs, trace=False):
    scenes, PP, in_maps, moving = _prepare(inputs)
    nc = _get_nc(scenes, PP)
    res = run_bass_kernel_spmd(nc, in_maps, core_ids=list(range(NCORES)),
                               trace=trace)
    # device pen includes the diagonal pairs (dist exactly 0 -> pen exactly
    # 1); their contribution per (i, n) is sum_t w_t/B = 1/B, gated by the
    # moving mask. Subtract it during unsharding.
    diag_corr = (1.0 / B) * moving.astype(np.float32)   # (B,)
    outf = np.zeros((B, N), dtype=np.float32)
    for c in range(NCORES):
        lc = res.results[c]["loss"]                      # (NL, B)
        for nl in range(NL):
            outf[:, c * NL + nl] = lc[nl] - diag_corr
    return outf, res


def kernel(**inputs):
    outf, _ = _run(inputs, trace=False)
    return outf


def _ensure_ntff_hook():
    """Register the axon NTFF profile hook if the container's antenv lacks it."""
    try:
        from antenv.axon_hooks import get_axon_ntff_profile_hook  # noqa: F401
        return
    except ImportError:
        pass
    import types

    if "/root/.axon_site" not in sys.path:
        sys.path.insert(0, "/root/.axon_site")
    from trn_agent_boot.trn_boot import _ntff_profile_via_ctypes

    hook = _ntff_profile_via_ctypes("/opt/axon/libaxon_pjrt.so")
    mod = types.ModuleType("antenv.axon_hooks")
    mod.get_axon_ntff_profile_hook = lambda: hook
    mod.set_axon_ntff_profile_hook = lambda h: None
    sys.modules["antenv.axon_hooks"] = mod


def run_traced(inputs):
    """Correctness output + profiled exec time (ns) via NTFF trace."""
    _ensure_ntff_hook()
    outf, res = _run(inputs, trace=True)
    return outf, res.exec_time_ns


# revision 25
# speedup vs baseline: 1.0685x; 1.0685x over previous
"""AgentCollisionLoss Trainium2 kernel.

Full inputs -> full output. Shards the N (sample) dim across 8 NeuronCores
(2 samples per core), computes the pairwise agent-collision loss on device,
and gathers the per-core (NL, B) losses into the full (B, N) output.

Device layout (per core):
  partition p = n_local*T + t            (104 rows)
  Stage A: world-frame disk centroids CXY [P, 2*B*D] from x + per-agent consts
  Stage B: per scene block, outer-difference over the packed disk-point list,
           squares on ACT, add + two-stage min-reduce over (dj, di) on DVE
  Stage C: sqrt, penalty = relu(1 - dist/pd) on the packed pair list
  Stage D: time-decay-weighted sum over t via a [P,2]^T @ [P,32] matmul,
           moving-mask, DMA out [2, 32]

All broadcast constants ride in the xin tensor (replicated per partition on
the host) so the kernel issues a single big input DMA.
"""

import os
import sys

import numpy as np

for _p in ("/opt/trn_rl_repo", "/root/.axon_site/_ro/trn_rl_repo"):
    if os.path.isdir(_p) and _p not in sys.path:
        sys.path.insert(0, _p)

import bass_rust
import concourse.bass as bass
import concourse.mybir as mybir
import concourse.tile as tile
from concourse.bass_utils import run_bass_kernel_spmd
from concourse.vector_clock import ScopedClock


def _split_drain_and_barrier(self, tick_clock, wait_clock):
    """Kernel-tail drain, one semaphore per drain instruction.

    The walrus build in this container rejects instructions carrying more
    than one embedded sync wait ("Too many sync wait commands"). Tile's
    stock tail emits a single drain waiting on the full global clock, so
    split it: one drain per nonzero proc tick. add_sem_waits elides waits
    the engine has already observed, so each drain carries exactly one.
    """
    gc = list(tick_clock.global_clock)
    for idx, tick in enumerate(gc):
        if tick <= 0:
            continue
        v = [0] * len(gc)
        v[idx] = tick
        d = self.nc.sync.drain()
        wait_clock.add_sem_waits(
            d.ins, ScopedClock({None: bass_rust.VectorClock(v)})
        )
    self.nc.all_engine_barrier()
    assert self.sems is not None
    popped = self.nc._tile_sem_poison_stack.pop()
    assert popped is self._sem_poison
    self.nc.clear_and_free_semaphores(list(self.sems.allocated().values()))
    self.nc.all_engine_barrier()


tile.TileContext._drain_and_barrier = _split_drain_and_barrier

B, N, T, D = 32, 16, 52, 5
NCORES = 8
NL = N // NCORES          # samples per core
P = NL * T                # partition rows per core
BUFFER_DIST = 0.2
DECAY_RATE = 0.9
SPEED_TH = 0.5
FMAX = 4000               # max free elems per big-stage chunk

F32 = mybir.dt.float32
F16 = mybir.dt.float16
PI = float(np.pi)

# bulk dtype for squared distances (precision analysed: d2 < 43000 < f16 max,
# and only d2 <= pd^2 ~ 16 matters, where f16 ulp ~ 0.008-0.016)
DT_BULK = F16
# gpsimd measured rates: 2-input fp16 add ~2.05 ns/elem, but broadcast-AP
# f32 subs ~3-3.6 ns/elem (worse than leaving them on DVE). So only the
# d2-adds of the non-largest chunks go to gpsimd.
SUB_ON_GPSIMD = ()
GPSIMD_ADD_MAX_F = 2100


def _chunks(scenes):
    """[(scene_off, ci, sc, s), ...] i-chunks, largest first."""
    out = []
    for (o, s) in scenes:
        sc_max = max(1, FMAX // (s * D * D))
        ci = 0
        while ci < s:
            sc = min(sc_max, s - ci)
            out.append((o, ci, sc, s))
            ci += sc
    out.sort(key=lambda c: -(c[2] * c[3]))
    return out


# xin column layout (offsets in f32 elements)
XO_X = 0                  # 3*B per-partition x data (x0 | x1 | yaw)
XO_GEO = 3 * B            # gA(2B) gB(2B) gT(2B) shifts2(2B) = 8B
XO_CENT = XO_GEO + 8 * B  # B*D
XO_MVR = XO_CENT + B * D  # 2B (moving mask, NL copies)
XO_PRC = XO_MVR + 2 * B   # PP
# then wmt (NL cols)


def _xin_width(PP):
    return XO_PRC + PP + NL


def _build_nc(scenes, PP):
    """Build the SPMD Bass program. `scenes` = [(offset, size)], PP = sum s^2."""
    nc = bass.Bass()

    XW = _xin_width(PP)
    xin = nc.dram_tensor("xin", [P, XW], F32, kind="ExternalInput")
    out = nc.dram_tensor("loss", [NL, B], F32, kind="ExternalOutput")

    chunks = _chunks(scenes)

    with tile.TileContext(nc) as tc:
        with (
            tc.tile_pool(name="singles", bufs=1) as singles,
            tc.tile_pool(name="small", bufs=1) as small,
            tc.tile_pool(name="big", bufs=1) as big,
            tc.tile_pool(name="psum", bufs=1, space="PSUM") as psum,
        ):
            # ---- loads ----
            xt = singles.tile([P, XW], F32)
            nc.sync.dma_start(out=xt[:], in_=xin[:])
            ones = singles.tile([P, 1], F32)
            nc.vector.memset(ones[:], 1.0)

            # Pre-touch the DMA'd tile on DVE: the copy carries the one
            # DMA-queue sem wait, so later compute ops joining DMA data with
            # engine-produced data need at most one new wait (this walrus
            # rejects instructions with more than one embedded sync wait).
            tch = singles.tile([P, 1], F32, tag="tch0")
            nc.vector.tensor_copy(out=tch[:], in_=xt[:, 0:1])
            # matmul weights via DVE so the PE matmul's deps are DVE-only
            wt2 = singles.tile([P, NL], F32)
            nc.vector.tensor_copy(out=wt2[:], in_=xt[:, XO_PRC + PP : XO_PRC + PP + NL])

            gA = xt[:, XO_GEO + 0 * B : XO_GEO + 2 * B]
            gB = xt[:, XO_GEO + 2 * B : XO_GEO + 4 * B]
            gT = xt[:, XO_GEO + 4 * B : XO_GEO + 6 * B]
            shifts2 = xt[:, XO_GEO + 6 * B : XO_GEO + 8 * B]
            x0 = xt[:, 0:B]
            x1 = xt[:, B : 2 * B]
            yw = xt[:, 2 * B : 3 * B]
            cxc = xt[:, XO_CENT : XO_CENT + B * D]
            movt = xt[0:NL, XO_MVR : XO_MVR + B]   # replicated const rows
            prc = xt[:, XO_PRC : XO_PRC + PP]

            def rep2(apx, w):
                """view [P, 2, w] reading apx's first w elems twice"""
                return bass.AP(tensor=apx.tensor, offset=apx.offset,
                               ap=[apx.ap[0], [0, 2], [1, w]])

            # ---- stage A ----
            # u = yaw/2pi + (shift + yoff/2pi)   (shift 2.0 -> sin, 2.25 -> cos)
            u2 = small.tile([P, 2, B], F32)
            nc.vector.scalar_tensor_tensor(
                out=u2[:], in0=rep2(yw, B), scalar=1.0 / (2.0 * PI),
                in1=shifts2.rearrange("p (c i) -> p c i", c=2),
                op0=mybir.AluOpType.mult, op1=mybir.AluOpType.add)
            # round-to-nearest-even via the 1.5*2^23 magic constant
            MAGIC = 12582912.0
            kf = small.tile([P, 2, B], F32)
            nc.vector.tensor_scalar(
                out=kf[:], in0=u2[:], scalar1=MAGIC, scalar2=MAGIC,
                op0=mybir.AluOpType.add, op1=mybir.AluOpType.subtract)
            fr = small.tile([P, 2, B], F32)
            nc.vector.tensor_sub(fr[:], u2[:], kf[:])
            # sincos[:, 0:32] = sin(yawg), [:, 32:64] = cos(yawg)
            sincos = small.tile([P, 2 * B], F32)
            nc.scalar.activation(out=sincos[:].rearrange("p (c i) -> p c i", c=2),
                                 in_=fr[:],
                                 func=mybir.ActivationFunctionType.Sin,
                                 bias=0.0, scale=2.0 * PI)

            # pos_g for both coords: pg[p, c, i], c=0 -> x, 1 -> y
            m1 = small.tile([P, 2, B], F32)
            m2 = small.tile([P, 2, B], F32)
            pg = small.tile([P, 2, B], F32)
            nc.vector.tensor_mul(m1[:], rep2(x0, B),
                                 gA.rearrange("p (c i) -> p c i", c=2))
            nc.vector.tensor_mul(m2[:], rep2(x1, B),
                                 gB.rearrange("p (c i) -> p c i", c=2))
            nc.vector.tensor_add(pg[:], m1[:], m2[:])
            nc.vector.tensor_add(pg[:], pg[:],
                                 gT.rearrange("p (c i) -> p c i", c=2))

            # CXY[p, c, i, di] = cent_x(i,di) * cs(c,i) + pg(c,i)
            # c=0 uses cos, c=1 uses sin (x = cx*cos + pgx, y = cx*sin + pgy)
            cxy = singles.tile([P, 2, B, D], F32)
            cs_sel = bass.AP(tensor=sincos.tensor, offset=sincos[:].offset + B,
                             ap=[sincos[:].ap[0], [-B, 2], [1, B], [0, D]])
            cx_rep = bass.AP(tensor=xt.tensor, offset=cxc.offset,
                             ap=[cxc.ap[0], [0, 2], [D, B], [1, D]])
            pg_bc = bass.AP(tensor=pg.tensor, offset=pg[:].offset,
                            ap=[pg[:].ap[0], [B, 2], [1, B], [0, D]])
            nc.vector.tensor_mul(cxy[:], cx_rep, cs_sel)
            nc.vector.tensor_add(cxy[:], cxy[:], pg_bc)

            cxyf = cxy[:].rearrange("p c i d -> p (c i d)")
            pap = cxyf.ap[0]
            e = cxyf.ap[-1][0]

            # ---- stage B ----
            pdist = singles.tile([P, PP], F32)
            NPTS = B * D

            def emit_subs(idx):
                (o, ci, sc, s) = chunks[idx]
                m, q = D * sc, D * s
                # one sub for both coords: (c, m, q) with A bcast over q,
                # B bcast over m
                a_ap = bass.AP(tensor=cxyf.tensor,
                               offset=cxyf.offset + (o + ci) * D * e,
                               ap=[pap, [NPTS * e, 2], [e, m], [0, q]])
                b_ap = bass.AP(tensor=cxyf.tensor,
                               offset=cxyf.offset + o * D * e,
                               ap=[pap, [NPTS * e, 2], [0, m], [e, q]])
                sub = big.tile([P, 2, m, q], DT_BULK, tag=f"sub{idx}")
                sub_eng = nc.gpsimd if idx in SUB_ON_GPSIMD else nc.vector
                sub_eng.tensor_tensor(out=sub[:], in0=a_ap, in1=b_ap,
                                      op=mybir.AluOpType.subtract)
                return sub

            def emit_squares(idx, sub):
                (o, ci, sc, s) = chunks[idx]
                m, q = D * sc, D * s
                # two squares (separate tiles keep the d2-add operands
                # tile-aligned for the fp16 2x mode)
                sq = {}
                for c, nm in ((0, "x"), (1, "y")):
                    tsq = big.tile([P, m, q], DT_BULK, tag=f"sq{nm}{idx}")
                    nc.scalar.activation(
                        out=tsq[:].rearrange("p a b -> p (a b)"),
                        in_=sub[:, c, :, :].rearrange("p a b -> p (a b)"),
                        func=mybir.ActivationFunctionType.Square)
                    sq[nm] = tsq
                return sq

            def emit_tail(idx, sq, poff):
                (o, ci, sc, s) = chunks[idx]
                m, q = D * sc, D * s
                d2 = big.tile([P, m, q], DT_BULK, tag=f"d2{idx}")
                add_eng = nc.gpsimd if m * q <= GPSIMD_ADD_MAX_F else nc.vector
                add_eng.tensor_tensor(
                    out=d2[:].rearrange("p a b -> p (a b)"),
                    in0=sq["x"][:].rearrange("p a b -> p (a b)"),
                    in1=sq["y"][:].rearrange("p a b -> p (a b)"),
                    op=mybir.AluOpType.add)
                # min over dj: view (m, j, dj), innermost dj; scatter-write
                # r1 in (i, j, di) order so the second reduce reads unit-stride
                r1 = big.tile([P, sc, s, D], DT_BULK, tag=f"r1{idx}")
                r1f = r1[:].rearrange("p a b c -> p (a b c)")
                e1 = r1f.ap[-1][0]
                r1scat = bass.AP(tensor=r1f.tensor, offset=r1f.offset,
                                 ap=[r1f.ap[0], [s * D * e1, sc], [e1, D],
                                     [D * e1, s]])
                nc.vector.tensor_reduce(
                    out=r1scat,
                    in_=d2[:].rearrange("p a (j dj) -> p a j dj", dj=D),
                    axis=mybir.AxisListType.X, op=mybir.AluOpType.min)
                pmin = pdist[:, poff : poff + sc * s].rearrange(
                    "p (a b) -> p a b", b=s)
                nc.vector.tensor_reduce(out=pmin, in_=r1[:],
                                        axis=mybir.AxisListType.X,
                                        op=mybir.AluOpType.min)

            # emission: all subs first (DVE and gpsimd queues fill in
            # parallel), then squares and tails in data-availability order
            # (DVE-sub chunks first, gpsimd-sub chunks after)
            poffs = []
            po = 0
            for (o, ci, sc, s) in chunks:
                poffs.append(po)
                po += sc * s
            assert po == PP
            order = [i for i in range(len(chunks)) if i not in SUB_ON_GPSIMD] \
                + [i for i in range(len(chunks)) if i in SUB_ON_GPSIMD]
            subs = {}
            for idx in range(len(chunks)):
                subs[idx] = emit_subs(idx)
            sqs = {}
            for idx in order:
                sqs[idx] = emit_squares(idx, subs[idx])
            for idx in order:
                emit_tail(idx, sqs[idx], poffs[idx])

            # ---- stage C ----
            dist = small.tile([P, PP], F32, tag="dist")
            nc.scalar.activation(out=dist[:], in_=pdist[:],
                                 func=mybir.ActivationFunctionType.Sqrt)
            rr = small.tile([P, PP], F32, tag="rr")
            nc.vector.tensor_mul(rr[:], dist[:], prc)
            # pen = relu(1 - r)
            pen = small.tile([P, PP], F32, tag="pen")
            nc.scalar.activation(out=pen[:], in_=rr[:],
                                 func=mybir.ActivationFunctionType.Relu,
                                 bias=ones[:], scale=-1.0)

            # ---- j-sums per chunk row-block -> loss32 [P, B] ----
            loss32 = singles.tile([P, B], F32)
            for idx, (o, ci, sc, s) in enumerate(chunks):
                pv = pen[:, poffs[idx] : poffs[idx] + sc * s].rearrange(
                    "p (a b) -> p a b", b=s)
                nc.vector.tensor_reduce(out=loss32[:, o + ci : o + ci + sc],
                                        in_=pv,
                                        axis=mybir.AxisListType.X,
                                        op=mybir.AluOpType.add)

            # ---- stage D ----
            ps = psum.tile([NL, B], F32)
            nc.tensor.matmul(ps[:], wt2[:], loss32[:], start=True, stop=True)
            lout = small.tile([NL, B], F32, tag="lout")
            nc.vector.tensor_mul(lout[:], ps[:], movt[:])
            nc.sync.dma_start(out=out[:], in_=lout[:])

    return nc


def _prepare(inputs):
    x = np.ascontiguousarray(inputs["x"], dtype=np.float32)
    extent = np.asarray(inputs["extent"], dtype=np.float32)
    wfa = np.asarray(inputs["world_from_agent"], dtype=np.float32)
    speed = np.asarray(inputs["curr_speed"], dtype=np.float32)
    scene = np.asarray(inputs["scene_index"])

    R = wfa[:, :2, :2]
    tr = wfa[:, :2, 2]
    yaw_off = np.arctan2(R[:, 1, 0], R[:, 0, 0]).astype(np.float32)
    agt_rad = extent[:, 1] / 2.0
    cent_min = -(extent[:, 0] / 2.0) + agt_rad
    cent_max = (extent[:, 0] / 2.0) - agt_rad
    lin = np.linspace(0.0, 1.0, D, dtype=np.float32)
    cent_x = (cent_min[:, None] + (cent_max - cent_min)[:, None] * lin).astype(
        np.float32)
    pd = (agt_rad[:, None] + agt_rad[None, :] + BUFFER_DIST).astype(np.float32)
    moving = (np.abs(speed) > SPEED_TH)

    # contiguous scene blocks (scene_index is sorted)
    _, starts, counts = np.unique(scene, return_index=True, return_counts=True)
    scenes = [(int(o), int(s)) for o, s in zip(starts, counts)]
    assert sum(s for _, s in scenes) == B
    for o, s in scenes:
        assert (scene[o : o + s] == scene[o]).all()

    chunks = _chunks(scenes)
    pairs_i = []
    pairs_j = []
    for (o, ci, sc, s) in chunks:
        for ii in range(o + ci, o + ci + sc):
            for jj in range(o, o + s):
                pairs_i.append(ii)
                pairs_j.append(jj)
    pairs_i = np.array(pairs_i)
    pairs_j = np.array(pairs_j)
    PP = len(pairs_i)
    inv_pd = (1.0 / pd[pairs_i, pairs_j]).astype(np.float32)

    twopi = 2.0 * np.pi
    geo = np.concatenate([
        R[:, 0, 0], R[:, 1, 0],          # gA
        R[:, 0, 1], R[:, 1, 1],          # gB
        tr[:, 0], tr[:, 1],              # gT
        2.0 + yaw_off / twopi, 2.25 + yaw_off / twopi,  # shifts2
    ]).astype(np.float32)

    w = DECAY_RATE ** np.arange(T, dtype=np.float32)
    w = w / w.sum()
    wmt = np.zeros((P, NL), dtype=np.float32)
    for nl in range(NL):
        wmt[nl * T : (nl + 1) * T, nl] = w / B

    # packed xin: per-partition x data + replicated consts + wmt
    XW = _xin_width(PP)
    mvr2 = np.tile(moving.astype(np.float32), NL)
    const_row = np.concatenate([geo, cent_x.reshape(-1), mvr2, inv_pd])
    in_maps = []
    for c in range(NCORES):
        xs = x[:, c * NL : (c + 1) * NL, :, :]          # (B, NL, T, 6)
        xs = xs[..., [0, 1, 3]]                          # (B, NL, T, 3)
        xdat = xs.transpose(1, 2, 3, 0).reshape(P, 3 * B)
        xin = np.empty((P, XW), dtype=np.float32)
        xin[:, 0 : 3 * B] = xdat
        xin[:, XO_GEO : XO_PRC + PP] = const_row[None, :]
        xin[:, XO_PRC + PP :] = wmt
        in_maps.append({"xin": xin})

    return scenes, PP, in_maps, moving


_CACHE = {}


def _get_nc(scenes, PP):
    key = (tuple(scenes), PP)
    if key not in _CACHE:
        _CACHE[key] = _build_nc(scenes, PP)
    return _CACHE[key]


def _run(inputs, trace=False):
    scenes, PP, in_maps, moving = _prepare(inputs)
    nc = _get_nc(scenes, PP)
    res = run_bass_kernel_spmd(nc, in_maps, core_ids=list(range(NCORES)),
                               trace=trace)
    # device pen includes the diagonal pairs (dist exactly 0 -> pen exactly
    # 1); their contribution per (i, n) is sum_t w_t/B = 1/B, gated by the
    # moving mask. Subtract it during unsharding.
    diag_corr = (1.0 / B) * moving.astype(np.float32)   # (B,)
    outf = np.zeros((B, N), dtype=np.float32)
    for c in range(NCORES):
        lc = res.results[c]["loss"]                      # (NL, B)
        for nl in range(NL):
            outf[:, c * NL + nl] = lc[nl] - diag_corr
    return outf, res


def kernel(**inputs):
    outf, _ = _run(inputs, trace=False)
    return outf


def _ensure_ntff_hook():
    """Register the axon NTFF profile hook if the container's antenv lacks it."""
    try:
        from antenv.axon_hooks import get_axon_ntff_profile_hook  # noqa: F401
        return
    except ImportError:
        pass
    import types

    if "/root/.axon_site" not in sys.path:
        sys.path.insert(0, "/root/.axon_site")
    from trn_agent_boot.trn_boot import _ntff_profile_via_ctypes

    hook = _ntff_profile_via_ctypes("/opt/axon/libaxon_pjrt.so")
    mod = types.ModuleType("antenv.axon_hooks")
    mod.get_axon_ntff_profile_hook = lambda: hook
    mod.set_axon_ntff_profile_hook = lambda h: None
    sys.modules["antenv.axon_hooks"] = mod


def run_traced(inputs):
    """Correctness output + profiled exec time (ns) via NTFF trace."""
    _ensure_ntff_hook()
    outf, res = _run(inputs, trace=True)
    return outf, res.exec_time_ns


# revision 26
# speedup vs baseline: 1.3388x; 1.2530x over previous
"""AgentCollisionLoss Trainium2 kernel.

Full inputs -> full output. Shards the N (sample) dim across 8 NeuronCores
(2 samples per core), computes the pairwise agent-collision loss on device,
and gathers the per-core (NL, B) losses into the full (B, N) output.

Device layout (per core):
  partition p = n_local*T + t            (104 rows)
  Stage A: world-frame disk centroids CXY [P, 2*B*D] from x + per-agent consts
  Stage B: per scene block, outer-difference over the packed disk-point list,
           squares on ACT, add + two-stage min-reduce over (dj, di) on DVE
  Stage C: sqrt, penalty = relu(1 - dist/pd) on the packed pair list
  Stage D: time-decay-weighted sum over t via a [P,2]^T @ [P,32] matmul,
           moving-mask, DMA out [2, 32]

All broadcast constants ride in the xin tensor (replicated per partition on
the host) so the kernel issues a single big input DMA.
"""

import os
import sys

import numpy as np

for _p in ("/opt/trn_rl_repo", "/root/.axon_site/_ro/trn_rl_repo"):
    if os.path.isdir(_p) and _p not in sys.path:
        sys.path.insert(0, _p)

import bass_rust
import concourse.bass as bass
import concourse.mybir as mybir
import concourse.tile as tile
from concourse.bass_utils import run_bass_kernel_spmd
from concourse.vector_clock import ScopedClock


def _split_drain_and_barrier(self, tick_clock, wait_clock):
    """Kernel-tail drain, one semaphore per drain instruction.

    The walrus build in this container rejects instructions carrying more
    than one embedded sync wait ("Too many sync wait commands"). Tile's
    stock tail emits a single drain waiting on the full global clock, so
    split it: one drain per nonzero proc tick. add_sem_waits elides waits
    the engine has already observed, so each drain carries exactly one.
    """
    gc = list(tick_clock.global_clock)
    for idx, tick in enumerate(gc):
        if tick <= 0:
            continue
        v = [0] * len(gc)
        v[idx] = tick
        d = self.nc.sync.drain()
        wait_clock.add_sem_waits(
            d.ins, ScopedClock({None: bass_rust.VectorClock(v)})
        )
    self.nc.all_engine_barrier()
    assert self.sems is not None
    popped = self.nc._tile_sem_poison_stack.pop()
    assert popped is self._sem_poison
    self.nc.clear_and_free_semaphores(list(self.sems.allocated().values()))
    self.nc.all_engine_barrier()


tile.TileContext._drain_and_barrier = _split_drain_and_barrier

B, N, T, D = 32, 16, 52, 5
NCORES = 8
NL = N // NCORES          # samples per core
P = NL * T                # partition rows per core
BUFFER_DIST = 0.2
DECAY_RATE = 0.9
SPEED_TH = 0.5
FMAX = 4000               # max free elems per big-stage chunk

F32 = mybir.dt.float32
F16 = mybir.dt.float16
PI = float(np.pi)

# bulk dtype for squared distances (precision analysed: d2 < 43000 < f16 max,
# and only d2 <= pd^2 ~ 16 matters, where f16 ulp ~ 0.008-0.016)
DT_BULK = F16
# gpsimd measured rates: 2-input fp16 add ~2.05 ns/elem, but broadcast-AP
# f32 subs ~3-3.6 ns/elem (worse than leaving them on DVE). So only the
# d2-adds of the non-largest chunks go to gpsimd.
SUB_ON_GPSIMD = ()
GPSIMD_ADD_MAX_F = 1300


def _chunks(scenes):
    """[(scene_off, ci, sc, s), ...] i-chunks, largest first."""
    out = []
    for (o, s) in scenes:
        sc_max = max(1, FMAX // (s * D * D))
        ci = 0
        while ci < s:
            sc = min(sc_max, s - ci)
            out.append((o, ci, sc, s))
            ci += sc
    out.sort(key=lambda c: -(c[2] * c[3]))
    return out


# xin column layout (offsets in f32 elements)
XO_X = 0                  # 3*B per-partition x data (x0 | x1 | yaw)
XO_GEO = 3 * B            # gA(2B) gB(2B) gT(2B) shifts2(2B) = 8B
XO_CENT = XO_GEO + 8 * B  # B*D
XO_MVR = XO_CENT + B * D  # 2B (moving mask, NL copies)
XO_PRC = XO_MVR + 2 * B   # PP
# then wmt (NL cols)


def _xin_width(PP):
    return XO_PRC + PP + NL


def _build_nc(scenes, PP):
    """Build the SPMD Bass program. `scenes` = [(offset, size)], PP = sum s^2."""
    nc = bass.Bass()

    XW = _xin_width(PP)
    xin = nc.dram_tensor("xin", [P, XW], F32, kind="ExternalInput")
    out = nc.dram_tensor("loss", [NL, B], F32, kind="ExternalOutput")

    chunks = _chunks(scenes)

    with tile.TileContext(nc) as tc:
        with (
            tc.tile_pool(name="singles", bufs=1) as singles,
            tc.tile_pool(name="small", bufs=1) as small,
            tc.tile_pool(name="big", bufs=1) as big,
            tc.tile_pool(name="psum", bufs=1, space="PSUM") as psum,
        ):
            # ---- loads ----
            xt = singles.tile([P, XW], F32)
            nc.sync.dma_start(out=xt[:], in_=xin[:])
            ones = singles.tile([P, 1], F32)
            nc.vector.memset(ones[:], 1.0)

            # Pre-touch the DMA'd tile on DVE: the copy carries the one
            # DMA-queue sem wait, so later compute ops joining DMA data with
            # engine-produced data need at most one new wait (this walrus
            # rejects instructions with more than one embedded sync wait).
            tch = singles.tile([P, 1], F32, tag="tch0")
            nc.vector.tensor_copy(out=tch[:], in_=xt[:, 0:1])
            # matmul weights via DVE so the PE matmul's deps are DVE-only
            wt2 = singles.tile([P, NL], F32)
            nc.vector.tensor_copy(out=wt2[:], in_=xt[:, XO_PRC + PP : XO_PRC + PP + NL])

            gA = xt[:, XO_GEO + 0 * B : XO_GEO + 2 * B]
            gB = xt[:, XO_GEO + 2 * B : XO_GEO + 4 * B]
            gT = xt[:, XO_GEO + 4 * B : XO_GEO + 6 * B]
            shifts2 = xt[:, XO_GEO + 6 * B : XO_GEO + 8 * B]
            x0 = xt[:, 0:B]
            x1 = xt[:, B : 2 * B]
            yw = xt[:, 2 * B : 3 * B]
            cxc = xt[:, XO_CENT : XO_CENT + B * D]
            movt = xt[0:NL, XO_MVR : XO_MVR + B]   # replicated const rows
            prc = xt[:, XO_PRC : XO_PRC + PP]

            def rep2(apx, w):
                """view [P, 2, w] reading apx's first w elems twice"""
                return bass.AP(tensor=apx.tensor, offset=apx.offset,
                               ap=[apx.ap[0], [0, 2], [1, w]])

            # ---- stage A ----
            # u = yaw/2pi + (shift + yoff/2pi)   (shift 2.0 -> sin, 2.25 -> cos)
            u2 = small.tile([P, 2, B], F32)
            nc.vector.scalar_tensor_tensor(
                out=u2[:], in0=rep2(yw, B), scalar=1.0 / (2.0 * PI),
                in1=shifts2.rearrange("p (c i) -> p c i", c=2),
                op0=mybir.AluOpType.mult, op1=mybir.AluOpType.add)
            # round-to-nearest-even via the 1.5*2^23 magic constant
            MAGIC = 12582912.0
            kf = small.tile([P, 2, B], F32)
            nc.vector.tensor_scalar(
                out=kf[:], in0=u2[:], scalar1=MAGIC, scalar2=MAGIC,
                op0=mybir.AluOpType.add, op1=mybir.AluOpType.subtract)
            fr = small.tile([P, 2, B], F32)
            nc.vector.tensor_sub(fr[:], u2[:], kf[:])
            # sincos[:, 0:32] = sin(yawg), [:, 32:64] = cos(yawg)
            sincos = small.tile([P, 2 * B], F32)
            nc.scalar.activation(out=sincos[:].rearrange("p (c i) -> p c i", c=2),
                                 in_=fr[:],
                                 func=mybir.ActivationFunctionType.Sin,
                                 bias=0.0, scale=2.0 * PI)

            # pos_g for both coords: pg[p, c, i], c=0 -> x, 1 -> y
            m1 = small.tile([P, 2, B], F32)
            m2 = small.tile([P, 2, B], F32)
            pg = small.tile([P, 2, B], F32)
            nc.vector.tensor_mul(m1[:], rep2(x0, B),
                                 gA.rearrange("p (c i) -> p c i", c=2))
            nc.vector.tensor_mul(m2[:], rep2(x1, B),
                                 gB.rearrange("p (c i) -> p c i", c=2))
            nc.vector.tensor_add(pg[:], m1[:], m2[:])
            nc.vector.tensor_add(pg[:], pg[:],
                                 gT.rearrange("p (c i) -> p c i", c=2))

            # CXY[p, c, i, di] = cent_x(i,di) * cs(c,i) + pg(c,i)
            # c=0 uses cos, c=1 uses sin (x = cx*cos + pgx, y = cx*sin + pgy)
            cxy = singles.tile([P, 2, B, D], F32)
            cs_sel = bass.AP(tensor=sincos.tensor, offset=sincos[:].offset + B,
                             ap=[sincos[:].ap[0], [-B, 2], [1, B], [0, D]])
            cx_rep = bass.AP(tensor=xt.tensor, offset=cxc.offset,
                             ap=[cxc.ap[0], [0, 2], [D, B], [1, D]])
            pg_bc = bass.AP(tensor=pg.tensor, offset=pg[:].offset,
                            ap=[pg[:].ap[0], [B, 2], [1, B], [0, D]])
            nc.vector.tensor_mul(cxy[:], cx_rep, cs_sel)
            nc.vector.tensor_add(cxy[:], cxy[:], pg_bc)

            cxyf = cxy[:].rearrange("p c i d -> p (c i d)")
            pap = cxyf.ap[0]
            e = cxyf.ap[-1][0]

            # ---- stage B ----
            pdist = singles.tile([P, PP], F32)
            NPTS = B * D

            def emit_subs(idx):
                (o, ci, sc, s) = chunks[idx]
                m, q = D * sc, D * s
                # one sub for both coords: (c, m, q) with A bcast over q,
                # B bcast over m
                a_ap = bass.AP(tensor=cxyf.tensor,
                               offset=cxyf.offset + (o + ci) * D * e,
                               ap=[pap, [NPTS * e, 2], [e, m], [0, q]])
                b_ap = bass.AP(tensor=cxyf.tensor,
                               offset=cxyf.offset + o * D * e,
                               ap=[pap, [NPTS * e, 2], [0, m], [e, q]])
                sub = big.tile([P, 2, m, q], DT_BULK, tag=f"sub{idx}")
                sub_eng = nc.gpsimd if idx in SUB_ON_GPSIMD else nc.vector
                sub_eng.tensor_tensor(out=sub[:], in0=a_ap, in1=b_ap,
                                      op=mybir.AluOpType.subtract)
                return sub

            def emit_squares(idx, sub):
                (o, ci, sc, s) = chunks[idx]
                m, q = D * sc, D * s
                # two squares (separate tiles keep the d2-add operands
                # tile-aligned for the fp16 2x mode)
                sq = {}
                for c, nm in ((0, "x"), (1, "y")):
                    tsq = big.tile([P, m, q], DT_BULK, tag=f"sq{nm}{idx}")
                    nc.scalar.activation(
                        out=tsq[:].rearrange("p a b -> p (a b)"),
                        in_=sub[:, c, :, :].rearrange("p a b -> p (a b)"),
                        func=mybir.ActivationFunctionType.Square)
                    sq[nm] = tsq
                return sq

            def emit_tail(idx, sq, poff):
                (o, ci, sc, s) = chunks[idx]
                m, q = D * sc, D * s
                d2 = big.tile([P, m, q], DT_BULK, tag=f"d2{idx}")
                add_eng = nc.gpsimd if m * q <= GPSIMD_ADD_MAX_F else nc.vector
                add_eng.tensor_tensor(
                    out=d2[:].rearrange("p a b -> p (a b)"),
                    in0=sq["x"][:].rearrange("p a b -> p (a b)"),
                    in1=sq["y"][:].rearrange("p a b -> p (a b)"),
                    op=mybir.AluOpType.add)
                # min over dj: view (m, j, dj), innermost dj; scatter-write
                # r1 in (i, j, di) order so the second reduce reads unit-stride
                r1 = big.tile([P, sc, s, D], DT_BULK, tag=f"r1{idx}")
                r1f = r1[:].rearrange("p a b c -> p (a b c)")
                e1 = r1f.ap[-1][0]
                r1scat = bass.AP(tensor=r1f.tensor, offset=r1f.offset,
                                 ap=[r1f.ap[0], [s * D * e1, sc], [e1, D],
                                     [D * e1, s]])
                nc.vector.tensor_reduce(
                    out=r1scat,
                    in_=d2[:].rearrange("p a (j dj) -> p a j dj", dj=D),
                    axis=mybir.AxisListType.X, op=mybir.AluOpType.min)
                pmin = pdist[:, poff : poff + sc * s].rearrange(
                    "p (a b) -> p a b", b=s)
                nc.vector.tensor_reduce(out=pmin, in_=r1[:],
                                        axis=mybir.AxisListType.X,
                                        op=mybir.AluOpType.min)

            # emission: all subs first (DVE and gpsimd queues fill in
            # parallel), then squares and tails in data-availability order
            # (DVE-sub chunks first, gpsimd-sub chunks after)
            poffs = []
            po = 0
            for (o, ci, sc, s) in chunks:
                poffs.append(po)
                po += sc * s
            assert po == PP
            order = [i for i in range(len(chunks)) if i not in SUB_ON_GPSIMD] \
                + [i for i in range(len(chunks)) if i in SUB_ON_GPSIMD]
            subs = {}
            for idx in range(len(chunks)):
                subs[idx] = emit_subs(idx)
            sqs = {}
            for idx in order:
                sqs[idx] = emit_squares(idx, subs[idx])
            for idx in order:
                emit_tail(idx, sqs[idx], poffs[idx])

            # ---- stage C ----
            dist = small.tile([P, PP], F32, tag="dist")
            nc.scalar.activation(out=dist[:], in_=pdist[:],
                                 func=mybir.ActivationFunctionType.Sqrt)
            rr = small.tile([P, PP], F32, tag="rr")
            nc.vector.tensor_mul(rr[:], dist[:], prc)
            # pen = relu(1 - r)
            pen = small.tile([P, PP], F32, tag="pen")
            nc.scalar.activation(out=pen[:], in_=rr[:],
                                 func=mybir.ActivationFunctionType.Relu,
                                 bias=ones[:], scale=-1.0)

            # ---- j-sums per chunk row-block -> loss32 [P, B] ----
            loss32 = singles.tile([P, B], F32)
            for idx, (o, ci, sc, s) in enumerate(chunks):
                pv = pen[:, poffs[idx] : poffs[idx] + sc * s].rearrange(
                    "p (a b) -> p a b", b=s)
                nc.vector.tensor_reduce(out=loss32[:, o + ci : o + ci + sc],
                                        in_=pv,
                                        axis=mybir.AxisListType.X,
                                        op=mybir.AluOpType.add)

            # ---- stage D ----
            ps = psum.tile([NL, B], F32)
            nc.tensor.matmul(ps[:], wt2[:], loss32[:], start=True, stop=True)
            lout = small.tile([NL, B], F32, tag="lout")
            nc.vector.tensor_mul(lout[:], ps[:], movt[:])
            nc.sync.dma_start(out=out[:], in_=lout[:])

    return nc


def _prepare(inputs):
    x = np.ascontiguousarray(inputs["x"], dtype=np.float32)
    extent = np.asarray(inputs["extent"], dtype=np.float32)
    wfa = np.asarray(inputs["world_from_agent"], dtype=np.float32)
    speed = np.asarray(inputs["curr_speed"], dtype=np.float32)
    scene = np.asarray(inputs["scene_index"])

    R = wfa[:, :2, :2]
    tr = wfa[:, :2, 2]
    yaw_off = np.arctan2(R[:, 1, 0], R[:, 0, 0]).astype(np.float32)
    agt_rad = extent[:, 1] / 2.0
    cent_min = -(extent[:, 0] / 2.0) + agt_rad
    cent_max = (extent[:, 0] / 2.0) - agt_rad
    lin = np.linspace(0.0, 1.0, D, dtype=np.float32)
    cent_x = (cent_min[:, None] + (cent_max - cent_min)[:, None] * lin).astype(
        np.float32)
    pd = (agt_rad[:, None] + agt_rad[None, :] + BUFFER_DIST).astype(np.float32)
    moving = (np.abs(speed) > SPEED_TH)

    # contiguous scene blocks (scene_index is sorted)
    _, starts, counts = np.unique(scene, return_index=True, return_counts=True)
    scenes = [(int(o), int(s)) for o, s in zip(starts, counts)]
    assert sum(s for _, s in scenes) == B
    for o, s in scenes:
        assert (scene[o : o + s] == scene[o]).all()

    chunks = _chunks(scenes)
    pairs_i = []
    pairs_j = []
    for (o, ci, sc, s) in chunks:
        for ii in range(o + ci, o + ci + sc):
            for jj in range(o, o + s):
                pairs_i.append(ii)
                pairs_j.append(jj)
    pairs_i = np.array(pairs_i)
    pairs_j = np.array(pairs_j)
    PP = len(pairs_i)
    inv_pd = (1.0 / pd[pairs_i, pairs_j]).astype(np.float32)

    twopi = 2.0 * np.pi
    geo = np.concatenate([
        R[:, 0, 0], R[:, 1, 0],          # gA
        R[:, 0, 1], R[:, 1, 1],          # gB
        tr[:, 0], tr[:, 1],              # gT
        2.0 + yaw_off / twopi, 2.25 + yaw_off / twopi,  # shifts2
    ]).astype(np.float32)

    w = DECAY_RATE ** np.arange(T, dtype=np.float32)
    w = w / w.sum()
    wmt = np.zeros((P, NL), dtype=np.float32)
    for nl in range(NL):
        wmt[nl * T : (nl + 1) * T, nl] = w / B

    # packed xin: per-partition x data + replicated consts + wmt
    XW = _xin_width(PP)
    mvr2 = np.tile(moving.astype(np.float32), NL)
    const_row = np.concatenate([geo, cent_x.reshape(-1), mvr2, inv_pd])
    in_maps = []
    for c in range(NCORES):
        xs = x[:, c * NL : (c + 1) * NL, :, :]          # (B, NL, T, 6)
        xs = xs[..., [0, 1, 3]]                          # (B, NL, T, 3)
        xdat = xs.transpose(1, 2, 3, 0).reshape(P, 3 * B)
        xin = np.empty((P, XW), dtype=np.float32)
        xin[:, 0 : 3 * B] = xdat
        xin[:, XO_GEO : XO_PRC + PP] = const_row[None, :]
        xin[:, XO_PRC + PP :] = wmt
        in_maps.append({"xin": xin})

    return scenes, PP, in_maps, moving


_CACHE = {}


def _get_nc(scenes, PP):
    key = (tuple(scenes), PP)
    if key not in _CACHE:
        _CACHE[key] = _build_nc(scenes, PP)
    return _CACHE[key]


def _run(inputs, trace=False):
    scenes, PP, in_maps, moving = _prepare(inputs)
    nc = _get_nc(scenes, PP)
    res = run_bass_kernel_spmd(nc, in_maps, core_ids=list(range(NCORES)),
                               trace=trace)
    # device pen includes the diagonal pairs (dist exactly 0 -> pen exactly
    # 1); their contribution per (i, n) is sum_t w_t/B = 1/B, gated by the
    # moving mask. Subtract it during unsharding.
    diag_corr = (1.0 / B) * moving.astype(np.float32)   # (B,)
    outf = np.zeros((B, N), dtype=np.float32)
    for c in range(NCORES):
        lc = res.results[c]["loss"]                      # (NL, B)
        for nl in range(NL):
            outf[:, c * NL + nl] = lc[nl] - diag_corr
    return outf, res


def kernel(**inputs):
    outf, _ = _run(inputs, trace=False)
    return outf


def _ensure_ntff_hook():
    """Register the axon NTFF profile hook if the container's antenv lacks it."""
    try:
        from antenv.axon_hooks import get_axon_ntff_profile_hook  # noqa: F401
        return
    except ImportError:
        pass
    import types

    if "/root/.axon_site" not in sys.path:
        sys.path.insert(0, "/root/.axon_site")
    from trn_agent_boot.trn_boot import _ntff_profile_via_ctypes

    hook = _ntff_profile_via_ctypes("/opt/axon/libaxon_pjrt.so")
    mod = types.ModuleType("antenv.axon_hooks")
    mod.get_axon_ntff_profile_hook = lambda: hook
    mod.set_axon_ntff_profile_hook = lambda h: None
    sys.modules["antenv.axon_hooks"] = mod


def run_traced(inputs):
    """Correctness output + profiled exec time (ns) via NTFF trace."""
    _ensure_ntff_hook()
    outf, res = _run(inputs, trace=True)
    return outf, res.exec_time_ns


# revision 28
# speedup vs baseline: 1.3642x; 1.0190x over previous
"""AgentCollisionLoss Trainium2 kernel.

Full inputs -> full output. Shards the N (sample) dim across 8 NeuronCores
(2 samples per core), computes the pairwise agent-collision loss on device,
and gathers the per-core (NL, B) losses into the full (B, N) output.

Device layout (per core):
  partition p = n_local*T + t            (104 rows)
  Stage A: world-frame disk centroids CXY [P, 2*B*D] from x + per-agent consts
  Stage B: per scene block, outer-difference over the packed disk-point list,
           squares on ACT, add + two-stage min-reduce over (dj, di) on DVE
  Stage C: sqrt, penalty = relu(1 - dist/pd) on the packed pair list
  Stage D: time-decay-weighted sum over t via a [P,2]^T @ [P,32] matmul,
           moving-mask, DMA out [2, 32]

All broadcast constants ride in the xin tensor (replicated per partition on
the host) so the kernel issues a single big input DMA.
"""

import os
import sys

import numpy as np

for _p in ("/opt/trn_rl_repo", "/root/.axon_site/_ro/trn_rl_repo"):
    if os.path.isdir(_p) and _p not in sys.path:
        sys.path.insert(0, _p)

import bass_rust
import concourse.bass as bass
import concourse.mybir as mybir
import concourse.tile as tile
from concourse.bass_utils import run_bass_kernel_spmd
from concourse.vector_clock import ScopedClock


def _split_drain_and_barrier(self, tick_clock, wait_clock):
    """Kernel-tail drain, one semaphore per drain instruction.

    The walrus build in this container rejects instructions carrying more
    than one embedded sync wait ("Too many sync wait commands"). Tile's
    stock tail emits a single drain waiting on the full global clock, so
    split it: one drain per nonzero proc tick. add_sem_waits elides waits
    the engine has already observed, so each drain carries exactly one.
    """
    gc = list(tick_clock.global_clock)
    for idx, tick in enumerate(gc):
        if tick <= 0:
            continue
        v = [0] * len(gc)
        v[idx] = tick
        d = self.nc.sync.drain()
        wait_clock.add_sem_waits(
            d.ins, ScopedClock({None: bass_rust.VectorClock(v)})
        )
    self.nc.all_engine_barrier()
    assert self.sems is not None
    popped = self.nc._tile_sem_poison_stack.pop()
    assert popped is self._sem_poison
    self.nc.clear_and_free_semaphores(list(self.sems.allocated().values()))
    self.nc.all_engine_barrier()


tile.TileContext._drain_and_barrier = _split_drain_and_barrier

B, N, T, D = 32, 16, 52, 5
NCORES = 8
NL = N // NCORES          # samples per core
P = NL * T                # partition rows per core
BUFFER_DIST = 0.2
DECAY_RATE = 0.9
SPEED_TH = 0.5
FMAX = 4000               # max free elems per big-stage chunk

F32 = mybir.dt.float32
F16 = mybir.dt.float16
PI = float(np.pi)

# bulk dtype for squared distances (precision analysed: d2 < 43000 < f16 max,
# and only d2 <= pd^2 ~ 16 matters, where f16 ulp ~ 0.008-0.016)
DT_BULK = F16
# gpsimd measured rates: 2-input fp16 add ~2.05 ns/elem, but broadcast-AP
# f32 subs ~3-3.6 ns/elem (worse than leaving them on DVE). So only the
# d2-adds of the non-largest chunks go to gpsimd.
SUB_ON_GPSIMD = ()
GPSIMD_ADD_MAX_F = 1300


def _chunks(scenes):
    """[(scene_off, ci, sc, s), ...] i-chunks, largest first."""
    out = []
    for (o, s) in scenes:
        sc_max = max(1, FMAX // (s * D * D))
        ci = 0
        while ci < s:
            sc = min(sc_max, s - ci)
            out.append((o, ci, sc, s))
            ci += sc
    out.sort(key=lambda c: -(c[2] * c[3]))
    return out


# xinA column layout (stage-A-critical): x(3B) | geo(8B) | cent(B*D)
XO_GEO = 3 * B
XO_CENT = XO_GEO + 8 * B
XWA = XO_CENT + B * D
# xinB column layout (stage-C): mvr(NL*B) | prc(PP) | wmt(NL)
XO_MVR = 0
XO_PRC = XO_MVR + NL * B


def _xin_width_b(PP):
    return XO_PRC + PP + NL


def _build_nc(scenes, PP):
    """Build the SPMD Bass program. `scenes` = [(offset, size)], PP = sum s^2."""
    nc = bass.Bass()

    XWB = _xin_width_b(PP)
    xina = nc.dram_tensor("xina", [P, XWA], F32, kind="ExternalInput")
    xinb = nc.dram_tensor("xinb", [P, XWB], F32, kind="ExternalInput")
    out = nc.dram_tensor("loss", [NL, B], F32, kind="ExternalOutput")

    chunks = _chunks(scenes)

    with tile.TileContext(nc) as tc:
        with (
            tc.tile_pool(name="singles", bufs=1) as singles,
            tc.tile_pool(name="small", bufs=1) as small,
            tc.tile_pool(name="big", bufs=1) as big,
            tc.tile_pool(name="psum", bufs=1, space="PSUM") as psum,
        ):
            # ---- loads (stage-A-critical part first) ----
            xta = singles.tile([P, XWA], F32)
            nc.sync.dma_start(out=xta[:], in_=xina[:])
            xtb = singles.tile([P, XWB], F32)
            nc.sync.dma_start(out=xtb[:], in_=xinb[:])
            ones = singles.tile([P, 1], F32)
            nc.vector.memset(ones[:], 1.0)

            # Pre-touch the DMA'd tiles on DVE: each copy carries one
            # DMA-queue sem wait, so later compute ops joining DMA data with
            # engine-produced data need at most one new wait (this walrus
            # rejects instructions with more than one embedded sync wait).
            tch = singles.tile([P, 1], F32, tag="tch0")
            nc.vector.tensor_copy(out=tch[:], in_=xta[:, 0:1])
            tchb = singles.tile([P, 1], F32, tag="tchb")
            nc.vector.tensor_copy(out=tchb[:], in_=xtb[:, 0:1])
            # matmul weights via DVE so the PE matmul's deps are DVE-only
            wt2 = singles.tile([P, NL], F32)
            nc.vector.tensor_copy(out=wt2[:], in_=xtb[:, XO_PRC + PP : XO_PRC + PP + NL])

            gA = xta[:, XO_GEO + 0 * B : XO_GEO + 2 * B]
            gB = xta[:, XO_GEO + 2 * B : XO_GEO + 4 * B]
            gT = xta[:, XO_GEO + 4 * B : XO_GEO + 6 * B]
            shifts2 = xta[:, XO_GEO + 6 * B : XO_GEO + 8 * B]
            x0 = xta[:, 0:B]
            x1 = xta[:, B : 2 * B]
            yw = xta[:, 2 * B : 3 * B]
            cxc = xta[:, XO_CENT : XO_CENT + B * D]
            movt = xtb[0:NL, XO_MVR : XO_MVR + B]   # replicated const rows
            prc = xtb[:, XO_PRC : XO_PRC + PP]

            def rep2(apx, w):
                """view [P, 2, w] reading apx's first w elems twice"""
                return bass.AP(tensor=apx.tensor, offset=apx.offset,
                               ap=[apx.ap[0], [0, 2], [1, w]])

            # ---- stage A ----
            # u = yaw/2pi + (shift + yoff/2pi)   (shift 2.0 -> sin, 2.25 -> cos)
            u2 = small.tile([P, 2, B], F32)
            nc.vector.scalar_tensor_tensor(
                out=u2[:], in0=rep2(yw, B), scalar=1.0 / (2.0 * PI),
                in1=shifts2.rearrange("p (c i) -> p c i", c=2),
                op0=mybir.AluOpType.mult, op1=mybir.AluOpType.add)
            # round-to-nearest-even via the 1.5*2^23 magic constant
            MAGIC = 12582912.0
            kf = small.tile([P, 2, B], F32)
            nc.vector.tensor_scalar(
                out=kf[:], in0=u2[:], scalar1=MAGIC, scalar2=MAGIC,
                op0=mybir.AluOpType.add, op1=mybir.AluOpType.subtract)
            fr = small.tile([P, 2, B], F32)
            nc.vector.tensor_sub(fr[:], u2[:], kf[:])
            # sincos[:, 0:32] = sin(yawg), [:, 32:64] = cos(yawg)
            sincos = small.tile([P, 2 * B], F32)
            nc.scalar.activation(out=sincos[:].rearrange("p (c i) -> p c i", c=2),
                                 in_=fr[:],
                                 func=mybir.ActivationFunctionType.Sin,
                                 bias=0.0, scale=2.0 * PI)

            # pos_g for both coords: pg[p, c, i], c=0 -> x, 1 -> y
            m1 = small.tile([P, 2, B], F32)
            m2 = small.tile([P, 2, B], F32)
            pg = small.tile([P, 2, B], F32)
            nc.vector.tensor_mul(m1[:], rep2(x0, B),
                                 gA.rearrange("p (c i) -> p c i", c=2))
            nc.vector.tensor_mul(m2[:], rep2(x1, B),
                                 gB.rearrange("p (c i) -> p c i", c=2))
            nc.vector.tensor_add(pg[:], m1[:], m2[:])
            nc.vector.tensor_add(pg[:], pg[:],
                                 gT.rearrange("p (c i) -> p c i", c=2))

            # CXY[p, c, i, di] = cent_x(i,di) * cs(c,i) + pg(c,i)
            # c=0 uses cos, c=1 uses sin (x = cx*cos + pgx, y = cx*sin + pgy)
            cxy = singles.tile([P, 2, B, D], F32)
            cs_sel = bass.AP(tensor=sincos.tensor, offset=sincos[:].offset + B,
                             ap=[sincos[:].ap[0], [-B, 2], [1, B], [0, D]])
            cx_rep = bass.AP(tensor=xta.tensor, offset=cxc.offset,
                             ap=[cxc.ap[0], [0, 2], [D, B], [1, D]])
            pg_bc = bass.AP(tensor=pg.tensor, offset=pg[:].offset,
                            ap=[pg[:].ap[0], [B, 2], [1, B], [0, D]])
            nc.vector.tensor_mul(cxy[:], cx_rep, cs_sel)
            nc.vector.tensor_add(cxy[:], cxy[:], pg_bc)

            cxyf = cxy[:].rearrange("p c i d -> p (c i d)")
            pap = cxyf.ap[0]
            e = cxyf.ap[-1][0]

            # ---- stage B ----
            pdist = singles.tile([P, PP], F32)
            NPTS = B * D

            def emit_subs(idx):
                (o, ci, sc, s) = chunks[idx]
                m, q = D * sc, D * s
                # one sub for both coords: (c, m, q) with A bcast over q,
                # B bcast over m
                a_ap = bass.AP(tensor=cxyf.tensor,
                               offset=cxyf.offset + (o + ci) * D * e,
                               ap=[pap, [NPTS * e, 2], [e, m], [0, q]])
                b_ap = bass.AP(tensor=cxyf.tensor,
                               offset=cxyf.offset + o * D * e,
                               ap=[pap, [NPTS * e, 2], [0, m], [e, q]])
                sub = big.tile([P, 2, m, q], DT_BULK, tag=f"sub{idx}")
                sub_eng = nc.gpsimd if idx in SUB_ON_GPSIMD else nc.vector
                sub_eng.tensor_tensor(out=sub[:], in0=a_ap, in1=b_ap,
                                      op=mybir.AluOpType.subtract)
                return sub

            def emit_squares(idx, sub):
                (o, ci, sc, s) = chunks[idx]
                m, q = D * sc, D * s
                # two squares (separate tiles keep the d2-add operands
                # tile-aligned for the fp16 2x mode)
                sq = {}
                for c, nm in ((0, "x"), (1, "y")):
                    tsq = big.tile([P, m, q], DT_BULK, tag=f"sq{nm}{idx}")
                    nc.scalar.activation(
                        out=tsq[:].rearrange("p a b -> p (a b)"),
                        in_=sub[:, c, :, :].rearrange("p a b -> p (a b)"),
                        func=mybir.ActivationFunctionType.Square)
                    sq[nm] = tsq
                return sq

            def emit_tail(idx, sq, poff):
                (o, ci, sc, s) = chunks[idx]
                m, q = D * sc, D * s
                d2 = big.tile([P, m, q], DT_BULK, tag=f"d2{idx}")
                add_eng = nc.gpsimd if m * q <= GPSIMD_ADD_MAX_F else nc.vector
                add_eng.tensor_tensor(
                    out=d2[:].rearrange("p a b -> p (a b)"),
                    in0=sq["x"][:].rearrange("p a b -> p (a b)"),
                    in1=sq["y"][:].rearrange("p a b -> p (a b)"),
                    op=mybir.AluOpType.add)
                # min over dj: view (m, j, dj), innermost dj; scatter-write
                # r1 in (i, j, di) order so the second reduce reads unit-stride
                r1 = big.tile([P, sc, s, D], DT_BULK, tag=f"r1{idx}")
                r1f = r1[:].rearrange("p a b c -> p (a b c)")
                e1 = r1f.ap[-1][0]
                r1scat = bass.AP(tensor=r1f.tensor, offset=r1f.offset,
                                 ap=[r1f.ap[0], [s * D * e1, sc], [e1, D],
                                     [D * e1, s]])
                nc.vector.tensor_reduce(
                    out=r1scat,
                    in_=d2[:].rearrange("p a (j dj) -> p a j dj", dj=D),
                    axis=mybir.AxisListType.X, op=mybir.AluOpType.min)
                pmin = pdist[:, poff : poff + sc * s].rearrange(
                    "p (a b) -> p a b", b=s)
                nc.vector.tensor_reduce(out=pmin, in_=r1[:],
                                        axis=mybir.AxisListType.X,
                                        op=mybir.AluOpType.min)

            # emission: all subs first (DVE and gpsimd queues fill in
            # parallel), then squares and tails in data-availability order
            # (DVE-sub chunks first, gpsimd-sub chunks after)
            poffs = []
            po = 0
            for (o, ci, sc, s) in chunks:
                poffs.append(po)
                po += sc * s
            assert po == PP
            order = [i for i in range(len(chunks)) if i not in SUB_ON_GPSIMD] \
                + [i for i in range(len(chunks)) if i in SUB_ON_GPSIMD]
            subs = {}
            for idx in range(len(chunks)):
                subs[idx] = emit_subs(idx)
            sqs = {}
            for idx in order:
                sqs[idx] = emit_squares(idx, subs[idx])
            for idx in order:
                emit_tail(idx, sqs[idx], poffs[idx])

            # ---- stage C ----
            dist = small.tile([P, PP], F32, tag="dist")
            nc.scalar.activation(out=dist[:], in_=pdist[:],
                                 func=mybir.ActivationFunctionType.Sqrt)
            rr = small.tile([P, PP], F32, tag="rr")
            nc.vector.tensor_mul(rr[:], dist[:], prc)
            # pen = relu(1 - r)
            pen = small.tile([P, PP], F32, tag="pen")
            nc.scalar.activation(out=pen[:], in_=rr[:],
                                 func=mybir.ActivationFunctionType.Relu,
                                 bias=ones[:], scale=-1.0)

            # ---- j-sums per chunk row-block -> loss32 [P, B] ----
            loss32 = singles.tile([P, B], F32)
            for idx, (o, ci, sc, s) in enumerate(chunks):
                pv = pen[:, poffs[idx] : poffs[idx] + sc * s].rearrange(
                    "p (a b) -> p a b", b=s)
                nc.vector.tensor_reduce(out=loss32[:, o + ci : o + ci + sc],
                                        in_=pv,
                                        axis=mybir.AxisListType.X,
                                        op=mybir.AluOpType.add)

            # ---- stage D ----
            ps = psum.tile([NL, B], F32)
            nc.tensor.matmul(ps[:], wt2[:], loss32[:], start=True, stop=True)
            lout = small.tile([NL, B], F32, tag="lout")
            nc.vector.tensor_mul(lout[:], ps[:], movt[:])
            nc.sync.dma_start(out=out[:], in_=lout[:])

    return nc


def _prepare(inputs):
    x = np.ascontiguousarray(inputs["x"], dtype=np.float32)
    extent = np.asarray(inputs["extent"], dtype=np.float32)
    wfa = np.asarray(inputs["world_from_agent"], dtype=np.float32)
    speed = np.asarray(inputs["curr_speed"], dtype=np.float32)
    scene = np.asarray(inputs["scene_index"])

    R = wfa[:, :2, :2]
    tr = wfa[:, :2, 2]
    yaw_off = np.arctan2(R[:, 1, 0], R[:, 0, 0]).astype(np.float32)
    agt_rad = extent[:, 1] / 2.0
    cent_min = -(extent[:, 0] / 2.0) + agt_rad
    cent_max = (extent[:, 0] / 2.0) - agt_rad
    lin = np.linspace(0.0, 1.0, D, dtype=np.float32)
    cent_x = (cent_min[:, None] + (cent_max - cent_min)[:, None] * lin).astype(
        np.float32)
    pd = (agt_rad[:, None] + agt_rad[None, :] + BUFFER_DIST).astype(np.float32)
    moving = (np.abs(speed) > SPEED_TH)

    # contiguous scene blocks (scene_index is sorted)
    _, starts, counts = np.unique(scene, return_index=True, return_counts=True)
    scenes = [(int(o), int(s)) for o, s in zip(starts, counts)]
    assert sum(s for _, s in scenes) == B
    for o, s in scenes:
        assert (scene[o : o + s] == scene[o]).all()

    chunks = _chunks(scenes)
    pairs_i = []
    pairs_j = []
    for (o, ci, sc, s) in chunks:
        for ii in range(o + ci, o + ci + sc):
            for jj in range(o, o + s):
                pairs_i.append(ii)
                pairs_j.append(jj)
    pairs_i = np.array(pairs_i)
    pairs_j = np.array(pairs_j)
    PP = len(pairs_i)
    inv_pd = (1.0 / pd[pairs_i, pairs_j]).astype(np.float32)

    twopi = 2.0 * np.pi
    geo = np.concatenate([
        R[:, 0, 0], R[:, 1, 0],          # gA
        R[:, 0, 1], R[:, 1, 1],          # gB
        tr[:, 0], tr[:, 1],              # gT
        2.0 + yaw_off / twopi, 2.25 + yaw_off / twopi,  # shifts2
    ]).astype(np.float32)

    w = DECAY_RATE ** np.arange(T, dtype=np.float32)
    w = w / w.sum()
    wmt = np.zeros((P, NL), dtype=np.float32)
    for nl in range(NL):
        wmt[nl * T : (nl + 1) * T, nl] = w / B

    # packed inputs: per-partition x data + replicated consts + wmt
    XWB = _xin_width_b(PP)
    mvr2 = np.tile(moving.astype(np.float32), NL)
    constA = np.concatenate([geo, cent_x.reshape(-1)])
    xinb_row = np.empty((P, XWB), dtype=np.float32)
    xinb_row[:, XO_MVR : XO_PRC + PP] = np.concatenate([mvr2, inv_pd])[None, :]
    xinb_row[:, XO_PRC + PP :] = wmt
    in_maps = []
    for c in range(NCORES):
        xs = x[:, c * NL : (c + 1) * NL, :, :]          # (B, NL, T, 6)
        xs = xs[..., [0, 1, 3]]                          # (B, NL, T, 3)
        xdat = xs.transpose(1, 2, 3, 0).reshape(P, 3 * B)
        xina = np.empty((P, XWA), dtype=np.float32)
        xina[:, 0 : 3 * B] = xdat
        xina[:, XO_GEO:] = constA[None, :]
        in_maps.append({"xina": xina, "xinb": xinb_row})

    return scenes, PP, in_maps, moving


_CACHE = {}


def _get_nc(scenes, PP):
    key = (tuple(scenes), PP)
    if key not in _CACHE:
        _CACHE[key] = _build_nc(scenes, PP)
    return _CACHE[key]


def _run(inputs, trace=False):
    scenes, PP, in_maps, moving = _prepare(inputs)
    nc = _get_nc(scenes, PP)
    res = run_bass_kernel_spmd(nc, in_maps, core_ids=list(range(NCORES)),
                               trace=trace)
    # device pen includes the diagonal pairs (dist exactly 0 -> pen exactly
    # 1); their contribution per (i, n) is sum_t w_t/B = 1/B, gated by the
    # moving mask. Subtract it during unsharding.
    diag_corr = (1.0 / B) * moving.astype(np.float32)   # (B,)
    outf = np.zeros((B, N), dtype=np.float32)
    for c in range(NCORES):
        lc = res.results[c]["loss"]                      # (NL, B)
        for nl in range(NL):
            outf[:, c * NL + nl] = lc[nl] - diag_corr
    return outf, res


def kernel(**inputs):
    outf, _ = _run(inputs, trace=False)
    return outf


def _ensure_ntff_hook():
    """Register the axon NTFF profile hook if the container's antenv lacks it."""
    try:
        from antenv.axon_hooks import get_axon_ntff_profile_hook  # noqa: F401
        return
    except ImportError:
        pass
    import types

    if "/root/.axon_site" not in sys.path:
        sys.path.insert(0, "/root/.axon_site")
    from trn_agent_boot.trn_boot import _ntff_profile_via_ctypes

    hook = _ntff_profile_via_ctypes("/opt/axon/libaxon_pjrt.so")
    mod = types.ModuleType("antenv.axon_hooks")
    mod.get_axon_ntff_profile_hook = lambda: hook
    mod.set_axon_ntff_profile_hook = lambda h: None
    sys.modules["antenv.axon_hooks"] = mod


def run_traced(inputs):
    """Correctness output + profiled exec time (ns) via NTFF trace."""
    _ensure_ntff_hook()
    outf, res = _run(inputs, trace=True)
    return outf, res.exec_time_ns
